# revision 11
# baseline (speedup 1.0000x reference)
"""Data-parallel Trainium2 kernel for the weighted classification loss.

loss = -mean_b sum_c w[b,c] * log(1 - softmax(reps @ W.T + b)[b,c])

Strategy (8 cores, batch-sharded 4096 rows each):
  - Host pre-casts reps to fp8e4 and pre-transposes into a matmul-ready
    [k-chunk x sample] layout; the kernel streams it HBM->SBUF with
    plain HWDGE DMAs (no on-chip cast/transpose).
  - Tapered chunks ([1024,1024,1024,512,256,256] samples): each chunk
    is an independent round whose matmuls start on its own DMA
    semaphore; the small final chunks shrink the post-stream tail.
  - Main matmul per chunk: K=128 fp8 chains over 8 D-chunks, 4-way
    column-tiled (tile_position=(0,32j), k-outer/j-inner) so 4
    sample-quarters accumulate concurrently into one PSUM tile as
    logits rows 32j..32j+9.
  - exp(l + bias) on ACT over the whole [128, Q] tile (4 groups at
    once); one diagonal-packed matmul vs a (ones - I | ones)-style
    stationary computes u_c = den - e_c (sum of positives) and den for
    all 4 groups; Ln on ACT; a host-prepared per-sample weight mask
    {0,1,2,-14} contracts w * ln(u) - 14*ln(den) via one DVE
    scalar_tensor_tensor with free-dim accumulate per chunk.
  - Per-chunk partial-sum columns DMA out as they finish (only the
    last sits on the critical tail); host combines.
"""

import os
import sys

import numpy as np

if "/opt/trn_rl_repo" not in sys.path:
    sys.path.insert(0, "/opt/trn_rl_repo")

import ml_dtypes

B, D, C = 32768, 1024, 10
NCORES = 8
SHARD = B // NCORES  # 4096
KCH = D // 128       # 8 contraction chunks
NGRP = 4
# chunk sizes in samples; each is one PSUM round of 4 column-groups.
# Finer taper: per-chunk DMA completion gates that chunk's matmuls, and
# the slowest DMA engine (the queue-owner, ~2x slower) piles chunks up on
# the PE; smaller chunks smooth the pile-up and shrink the post-stream
# tail to the last 128-sample chunk's chain.
CHUNKS = [512, 512, 512, 512, 384, 384, 320, 256, 256, 192, 128, 128]
assert sum(CHUNKS) == SHARD and all(s % NGRP == 0 for s in CHUNKS)
OFFS = [sum(CHUNKS[:i]) for i in range(len(CHUNKS))]
NCHK = len(CHUNKS)
MID = 5
OPP_W = 2.0

_CACHE: dict = {}


def _build_nc():
    from contextlib import ExitStack

    import concourse.mybir as mybir
    import concourse.tile as tile
    from concourse import bacc
    from concourse.tile import add_dep_helper

    f32 = mybir.dt.float32
    bf16 = mybir.dt.bfloat16
    fp8 = mybir.dt.float8e4
    Exp = mybir.ActivationFunctionType.Exp
    Ln = mybir.ActivationFunctionType.Ln
    alu = mybir.AluOpType

    nc = bacc.Bacc(
        "TRN2",
        target_bir_lowering=False,
        debug=False,
        enable_asserts=False,
        num_devices=NCORES,
    )
    u8 = mybir.dt.uint8
    repsq = nc.dram_tensor("repsq", [128, KCH * SHARD], fp8,
                           kind="ExternalInput").ap()
    # all consts packed into one byte tensor so ONE DMA (one ~0.8us issue,
    # one semaphore) carries them, ahead of the input stream on the same
    # ring: bytes [0:2112] cpack bf16 (uzw4 cols 0..31 + weight-mask),
    # [2112:2192] wq fp8, [2192:2196] bias4 f32
    CPACK_B = 2 * (32 + SHARD // NGRP)
    WQ_B = KCH * C
    CONST_B = CPACK_B + WQ_B + 4
    consts = nc.dram_tensor("consts", [128, CONST_B], u8,
                            kind="ExternalInput").ap()
    partials = nc.dram_tensor("partials", [128, NCHK], f32,
                              kind="ExternalOutput").ap()

    with tile.TileContext(nc) as tc:
        with ExitStack() as ctx:
            const_pool = ctx.enter_context(tc.tile_pool(name="const", bufs=1))
            sb_pool = ctx.enter_context(tc.tile_pool(name="sb", bufs=3))
            lp_pool = ctx.enter_context(
                tc.tile_pool(name="lp", bufs=3, space="PSUM"))
            u_pool = ctx.enter_context(
                tc.tile_pool(name="u", bufs=2, space="PSUM"))

            # single const DMA FIRST on the SP ring: lands before chunk0
            # finishes streaming, so the matmuls (wq), EXP (bias) and STT
            # (mask) are never gated on a trickling second-ring transfer
            cn_t = const_pool.tile([128, CONST_B], u8, tag="consts")
            nc.sync.dma_start(cn_t[:], consts)

            # Pin the combined exp+ln activation table (set 6:
            # natural_log_exp_and_others) once, up front, so the compiler's
            # per-function table placement doesn't ping-pong reloads.
            ld_tab = nc.scalar.add_instruction(
                mybir.InstLoadActFuncSet(
                    name=nc.get_next_instruction_name(),
                    ins=[],
                    outs=[],
                    act_func_set_id=6,
                )
            )

            # input chunks on the SP ring behind the consts (FIFO => chunk
            # c completes at ~its share of the stream); per-chunk tiles
            # keep Tile's DMA->matmul deps per-chunk
            xb = []
            for c, (off, sz) in enumerate(zip(OFFS, CHUNKS)):
                t = const_pool.tile([128, KCH * sz], fp8, tag=f"x{c}")
                nc.sync.dma_start(
                    t[:], repsq[:, KCH * off : KCH * (off + sz)]
                )
                xb.append(t[:].rearrange("p (k m) -> p k m", k=KCH))
            cp_t = cn_t[:, 0:CPACK_B].bitcast(bf16)
            wq_t = cn_t[:, CPACK_B : CPACK_B + WQ_B].bitcast(fp8)
            bias_t = cn_t[:, CPACK_B + WQ_B : CONST_B].bitcast(f32)
            uzw_t = cp_t[:, 0:32]
            mask_t = cp_t[:, 32:]
            acc = const_pool.tile([128, NCHK], f32, tag="acc")
            wv = wq_t.rearrange("p (k c) -> p k c", k=KCH)

            lp_tiles = {}
            first_act = None

            def emit_mains(r):
                q = CHUNKS[r] // NGRP
                lp = lp_pool.tile([128, q], f32, tag="lp", name=f"lp{r}")
                lp_tiles[r] = lp
                # k-outer / j-inner: adjacent MMs hit different col-groups
                # so all 4 stream concurrently (MATMUL issue is strict FIFO)
                for k in range(KCH):
                    for j in range(NGRP):
                        nc.tensor.matmul(
                            lp[32 * j : 32 * j + C, :],
                            wv[:, k, :],
                            xb[r][:, k, j * q : (j + 1) * q],
                            start=(k == 0),
                            stop=(k == KCH - 1),
                            skip_group_check=True,
                            tile_position=(0, 32 * j),
                        )

            def emit_tail(r):
                nonlocal first_act
                q = CHUNKS[r] // NGRP
                moff = OFFS[r] // NGRP
                lp = lp_tiles.pop(r)
                e = sb_pool.tile([128, q], bf16, tag="e", name=f"e{r}")
                act = nc.scalar.activation(
                    e[:], lp[:], Exp, bias=bias_t, scale=1.0
                )
                if first_act is None:
                    first_act = act
                    add_dep_helper(
                        act.ins, ld_tab.ins, sync=False,
                        reason="combined exp+ln table pinned before first ACT",
                    )

                u = u_pool.tile([128, q], f32, tag="u", name=f"u{r}")
                for j in range(NGRP):
                    nc.tensor.matmul(
                        u[32 * j : 32 * j + 32, :],
                        uzw_t[32 * j : 32 * j + C, :],
                        e[32 * j : 32 * j + C, :],
                        start=True,
                        stop=True,
                        skip_group_check=True,
                        tile_position=(32 * j, 32 * j),
                    )

                lnu = sb_pool.tile([128, q], bf16, tag="lnu", name=f"ln{r}")
                nc.scalar.activation(lnu[:], u[:], Ln)

                scr = sb_pool.tile([128, q], f32, tag="scr", name=f"sc{r}")
                nc.vector.scalar_tensor_tensor(
                    out=scr[:],
                    in0=mask_t[:, moff : moff + q],
                    scalar=1.0,
                    in1=lnu[:],
                    op0=alu.mult,
                    op1=alu.mult,
                    accum_out=acc[:, r : r + 1],
                )

            # software-pipelined: round r's tail is emitted after round
            # r+1's matmuls so the PE never stalls waiting on ACT
            emit_mains(0)
            for r in range(1, NCHK):
                emit_mains(r)
                emit_tail(r - 1)
            emit_tail(NCHK - 1)

            # two output DMAs: the first NCHK-1 accumulated columns go out
            # once chunk NCHK-2's STT lands (off the critical tail); only
            # the last 4B column's issue+flight trails the final STT
            nc.sync.dma_start(partials[:, 0 : NCHK - 1],
                              acc[:, 0 : NCHK - 1])
            nc.sync.dma_start(partials[:, NCHK - 1 : NCHK],
                              acc[:, NCHK - 1 : NCHK])

    nc.compile()
    return nc


def _prepare_static(W: np.ndarray, b: np.ndarray):
    # wq[p, k*C + c] = fp8(W[c, 128k + p])
    wq = np.zeros((128, KCH * C), dtype=np.float32)
    for k in range(KCH):
        wq[:, k * C : (k + 1) * C] = W[:, k * 128 : (k + 1) * 128].T
    wq = wq.astype(ml_dtypes.float8_e4m3)

    # u = uzw_ext.T @ e per group: cols 0..9 -> den - e_c (sum of
    # positives), cols 10..31 -> den (keeps every PSUM row defined > 0)
    uzw_ext = np.ones((C, 32), dtype=np.float32)
    uzw_ext[:, :C] -= np.eye(C, dtype=np.float32)
    uzw4 = np.zeros((128, 32), dtype=np.float32)
    for j in range(NGRP):
        uzw4[32 * j : 32 * j + C, :] = uzw_ext

    bias4 = np.zeros((128, 1), dtype=np.float32)
    for j in range(NGRP):
        bias4[32 * j : 32 * j + C, 0] = b
    return wq, uzw4, bias4


def _prepare_cpack(uzw4: np.ndarray, labels_sh: np.ndarray) -> np.ndarray:
    """cpack[:, 0:32] = uzw4; mask column layout mirrors the on-chip
    per-chunk group quarters: chunk r, group j, n -> sample
    OFFS[r] + j*q + n at mask[32j + c, 32 + OFFS[r]//4 + n]."""
    cc = np.arange(C).reshape(1, C)
    m = np.zeros((128, SHARD // NGRP), dtype=np.float32)
    for r, (off, sz) in enumerate(zip(OFFS, CHUNKS)):
        q = sz // NGRP
        moff = off // NGRP
        for j in range(NGRP):
            lab = labels_sh[off + j * q : off + (j + 1) * q].astype(np.int64)
            ll = lab.reshape(q, 1)
            opp = (cc < MID) != (ll < MID)
            w = np.where(cc == ll, 0.0, np.where(opp, OPP_W, 1.0))  # [q, C]
            m[32 * j : 32 * j + C, moff : moff + q] = w.T
            m[32 * j + C, moff : moff + q] = -float(C + MID - 1)
    cp = np.concatenate([uzw4, m], axis=1)
    return cp.astype(ml_dtypes.bfloat16)


def _pack_consts(cpack_bf16, wq_fp8, bias4_f32) -> np.ndarray:
    """One [128, CPACK_B+WQ_B+4] uint8 tensor: cpack | wq | bias4."""
    out = np.concatenate(
        [
            np.ascontiguousarray(cpack_bf16).view(np.uint8),
            np.ascontiguousarray(wq_fp8).view(np.uint8),
            np.ascontiguousarray(bias4_f32).view(np.uint8),
        ],
        axis=1,
    )
    return out


def _prepare_reps(reps_sh: np.ndarray) -> np.ndarray:
    """repsq[p, KCH*off + k*sz + m] = fp8(reps_sh[off + m, 128k + p])
    for each chunk (off, sz)."""
    out = np.empty((128, KCH * SHARD), dtype=ml_dtypes.float8_e4m3)
    for off, sz in zip(OFFS, CHUNKS):
        x = reps_sh[off : off + sz].astype(ml_dtypes.float8_e4m3)
        x = x.reshape(sz, KCH, 128)                 # [m, k, p]
        x = np.ascontiguousarray(x.transpose(2, 1, 0))  # [p, k, m]
        out[:, KCH * off : KCH * (off + sz)] = x.reshape(128, KCH * sz)
    return out


def kernel(reps, W, b, labels):
    from concourse.bass_utils import run_bass_kernel_spmd

    reps = np.asarray(reps, dtype=np.float32)
    W = np.asarray(W, dtype=np.float32)
    b = np.asarray(b, dtype=np.float32)
    labels_np = np.asarray(labels)

    if "nc" not in _CACHE:
        _CACHE["nc"] = _build_nc()
    nc = _CACHE["nc"]

    wq, uzw4, bias4 = _prepare_static(W, b)

    in_maps = []
    for core in range(NCORES):
        sh = slice(core * SHARD, (core + 1) * SHARD)
        in_maps.append(
            {
                "repsq": _prepare_reps(reps[sh]),
                "consts": _pack_consts(
                    _prepare_cpack(uzw4, labels_np[sh]), wq, bias4
                ),
            }
        )

    trace = bool(int(os.environ.get("CC_KERNEL_TRACE", "0")))
    res = run_bass_kernel_spmd(
        nc, in_maps, core_ids=list(range(NCORES)), trace=trace
    )
    if trace:
        _CACHE["last_results"] = res

    total = np.float64(0.0)
    for core in range(NCORES):
        total += np.float64(res.results[core]["partials"].sum(dtype=np.float64))
    loss = -(total / B)
    return np.float32(loss)



# revision 15
# speedup vs baseline: 1.1518x; 1.1518x over previous
"""Data-parallel Trainium2 kernel for the weighted classification loss.

loss = -mean_b sum_c w[b,c] * log(1 - softmax(reps @ W.T + b)[b,c])

Strategy (8 cores, batch-sharded 4096 rows each):
  - Host pre-casts reps to fp8e4 and pre-transposes into a matmul-ready
    [k-chunk x sample] layout; the kernel streams it HBM->SBUF with
    plain HWDGE DMAs (no on-chip cast/transpose).
  - Tapered chunks ([1024,1024,1024,512,256,256] samples): each chunk
    is an independent round whose matmuls start on its own DMA
    semaphore; the small final chunks shrink the post-stream tail.
  - Main matmul per chunk: K=128 fp8 chains over 8 D-chunks, 4-way
    column-tiled (tile_position=(0,32j), k-outer/j-inner) so 4
    sample-quarters accumulate concurrently into one PSUM tile as
    logits rows 32j..32j+9.
  - exp(l + bias) on ACT over the whole [128, Q] tile (4 groups at
    once); one diagonal-packed matmul vs a (ones - I | ones)-style
    stationary computes u_c = den - e_c (sum of positives) and den for
    all 4 groups; Ln on ACT; a host-prepared per-sample weight mask
    {0,1,2,-14} contracts w * ln(u) - 14*ln(den) via one DVE
    scalar_tensor_tensor with free-dim accumulate per chunk.
  - Per-chunk partial-sum columns DMA out as they finish (only the
    last sits on the critical tail); host combines.
"""

import os
import sys

import numpy as np

if "/opt/trn_rl_repo" not in sys.path:
    sys.path.insert(0, "/opt/trn_rl_repo")

import ml_dtypes

B, D, C = 32768, 1024, 10
NCORES = 8
SHARD = B // NCORES  # 4096
KCH = D // 128       # 8 contraction chunks
NGRP = 4
# chunk sizes in samples; each is one PSUM round of 4 column-groups.
# The PE FIFO costs ~27ns/instruction, so a chunk below ~1024 samples is
# issue-bound at ~1.9us regardless of size — tapering below 1024 only
# adds instructions. Uniform 1024 chunks minimize total PE work; the
# post-stream tail is one chunk's mains (~2.1us) + its ACT/DVE chain.
CHUNKS = [1024, 1024, 1024, 1024]
assert sum(CHUNKS) == SHARD and all(s % NGRP == 0 for s in CHUNKS)
OFFS = [sum(CHUNKS[:i]) for i in range(len(CHUNKS))]
NCHK = len(CHUNKS)
MID = 5
OPP_W = 2.0

_CACHE: dict = {}


def _build_nc():
    from contextlib import ExitStack

    import concourse.mybir as mybir
    import concourse.tile as tile
    from concourse import bacc
    from concourse.tile import add_dep_helper

    f32 = mybir.dt.float32
    bf16 = mybir.dt.bfloat16
    fp8 = mybir.dt.float8e4
    Exp = mybir.ActivationFunctionType.Exp
    Ln = mybir.ActivationFunctionType.Ln
    alu = mybir.AluOpType

    nc = bacc.Bacc(
        "TRN2",
        target_bir_lowering=False,
        debug=False,
        enable_asserts=False,
        num_devices=NCORES,
    )
    u8 = mybir.dt.uint8
    repsq = nc.dram_tensor("repsq", [128, KCH * SHARD], fp8,
                           kind="ExternalInput").ap()
    # all consts packed into one byte tensor so ONE DMA (one ~0.8us issue,
    # one semaphore) carries them, ahead of the input stream on the same
    # ring: bytes [0:2112] cpack bf16 (uzw4 cols 0..31 + weight-mask),
    # [2112:2192] wq fp8, [2192:2196] bias4 f32
    CPACK_B = 2 * (32 + SHARD // NGRP)
    WQ_B = KCH * C
    CONST_B = CPACK_B + WQ_B + 4
    consts = nc.dram_tensor("consts", [128, CONST_B], u8,
                            kind="ExternalInput").ap()
    partials = nc.dram_tensor("partials", [128, NCHK], f32,
                              kind="ExternalOutput").ap()

    with tile.TileContext(nc) as tc:
        with ExitStack() as ctx:
            const_pool = ctx.enter_context(tc.tile_pool(name="const", bufs=1))
            sb_pool = ctx.enter_context(tc.tile_pool(name="sb", bufs=3))
            lp_pool = ctx.enter_context(
                tc.tile_pool(name="lp", bufs=3, space="PSUM"))
            u_pool = ctx.enter_context(
                tc.tile_pool(name="u", bufs=2, space="PSUM"))

            # single const DMA FIRST on the SP ring: lands before chunk0
            # finishes streaming, so the matmuls (wq), EXP (bias) and STT
            # (mask) are never gated on a trickling second-ring transfer
            cn_t = const_pool.tile([128, CONST_B], u8, tag="consts")
            nc.sync.dma_start(cn_t[:], consts)

            # Pin the combined exp+ln activation table (set 6:
            # natural_log_exp_and_others) once, up front, so the compiler's
            # per-function table placement doesn't ping-pong reloads.
            ld_tab = nc.scalar.add_instruction(
                mybir.InstLoadActFuncSet(
                    name=nc.get_next_instruction_name(),
                    ins=[],
                    outs=[],
                    act_func_set_id=6,
                )
            )

            # input chunks on the SP ring behind the consts (FIFO => chunk
            # c completes at ~its share of the stream); per-chunk tiles
            # keep Tile's DMA->matmul deps per-chunk
            xb = []
            for c, (off, sz) in enumerate(zip(OFFS, CHUNKS)):
                t = const_pool.tile([128, KCH * sz], fp8, tag=f"x{c}")
                nc.sync.dma_start(
                    t[:], repsq[:, KCH * off : KCH * (off + sz)]
                )
                xb.append(t[:].rearrange("p (k m) -> p k m", k=KCH))
            cp_t = cn_t[:, 0:CPACK_B].bitcast(bf16)
            wq_t = cn_t[:, CPACK_B : CPACK_B + WQ_B].bitcast(fp8)
            bias_t = cn_t[:, CPACK_B + WQ_B : CONST_B].bitcast(f32)
            uzw_t = cp_t[:, 0:32]
            mask_t = cp_t[:, 32:]
            acc = const_pool.tile([128, NCHK], f32, tag="acc")
            wv = wq_t.rearrange("p (k c) -> p k c", k=KCH)

            lp_tiles = {}
            first_act = None

            def emit_mains(r):
                q = CHUNKS[r] // NGRP
                lp = lp_pool.tile([128, q], f32, tag="lp", name=f"lp{r}")
                lp_tiles[r] = lp
                # k-outer / j-inner: adjacent MMs hit different col-groups
                # so all 4 stream concurrently (MATMUL issue is strict FIFO)
                for k in range(KCH):
                    for j in range(NGRP):
                        nc.tensor.matmul(
                            lp[32 * j : 32 * j + C, :],
                            wv[:, k, :],
                            xb[r][:, k, j * q : (j + 1) * q],
                            start=(k == 0),
                            stop=(k == KCH - 1),
                            skip_group_check=True,
                            tile_position=(0, 32 * j),
                        )

            def emit_tail(r):
                nonlocal first_act
                q = CHUNKS[r] // NGRP
                moff = OFFS[r] // NGRP
                lp = lp_tiles.pop(r)
                e = sb_pool.tile([128, q], bf16, tag="e", name=f"e{r}")
                act = nc.scalar.activation(
                    e[:], lp[:], Exp, bias=bias_t, scale=1.0
                )
                if first_act is None:
                    first_act = act
                    add_dep_helper(
                        act.ins, ld_tab.ins, sync=False,
                        reason="combined exp+ln table pinned before first ACT",
                    )

                u = u_pool.tile([128, q], f32, tag="u", name=f"u{r}")
                for j in range(NGRP):
                    nc.tensor.matmul(
                        u[32 * j : 32 * j + 32, :],
                        uzw_t[32 * j : 32 * j + C, :],
                        e[32 * j : 32 * j + C, :],
                        start=True,
                        stop=True,
                        skip_group_check=True,
                        tile_position=(32 * j, 32 * j),
                    )

                lnu = sb_pool.tile([128, q], bf16, tag="lnu", name=f"ln{r}")
                nc.scalar.activation(lnu[:], u[:], Ln)

                scr = sb_pool.tile([128, q], f32, tag="scr", name=f"sc{r}")
                nc.vector.scalar_tensor_tensor(
                    out=scr[:],
                    in0=mask_t[:, moff : moff + q],
                    scalar=1.0,
                    in1=lnu[:],
                    op0=alu.mult,
                    op1=alu.mult,
                    accum_out=acc[:, r : r + 1],
                )

            # software-pipelined: round r's tail is emitted after round
            # r+1's matmuls so the PE never stalls waiting on ACT
            emit_mains(0)
            for r in range(1, NCHK):
                emit_mains(r)
                emit_tail(r - 1)
            emit_tail(NCHK - 1)

            # two output DMAs: the first NCHK-1 accumulated columns go out
            # once chunk NCHK-2's STT lands (off the critical tail); only
            # the last 4B column's issue+flight trails the final STT
            nc.sync.dma_start(partials[:, 0 : NCHK - 1],
                              acc[:, 0 : NCHK - 1])
            nc.sync.dma_start(partials[:, NCHK - 1 : NCHK],
                              acc[:, NCHK - 1 : NCHK])

    nc.compile()
    return nc


def _prepare_static(W: np.ndarray, b: np.ndarray):
    # wq[p, k*C + c] = fp8(W[c, 128k + p])
    wq = np.zeros((128, KCH * C), dtype=np.float32)
    for k in range(KCH):
        wq[:, k * C : (k + 1) * C] = W[:, k * 128 : (k + 1) * 128].T
    wq = wq.astype(ml_dtypes.float8_e4m3)

    # u = uzw_ext.T @ e per group: cols 0..9 -> den - e_c (sum of
    # positives), cols 10..31 -> den (keeps every PSUM row defined > 0)
    uzw_ext = np.ones((C, 32), dtype=np.float32)
    uzw_ext[:, :C] -= np.eye(C, dtype=np.float32)
    uzw4 = np.zeros((128, 32), dtype=np.float32)
    for j in range(NGRP):
        uzw4[32 * j : 32 * j + C, :] = uzw_ext

    bias4 = np.zeros((128, 1), dtype=np.float32)
    for j in range(NGRP):
        bias4[32 * j : 32 * j + C, 0] = b
    return wq, uzw4, bias4


def _prepare_cpack(uzw4: np.ndarray, labels_sh: np.ndarray) -> np.ndarray:
    """cpack[:, 0:32] = uzw4; mask column layout mirrors the on-chip
    per-chunk group quarters: chunk r, group j, n -> sample
    OFFS[r] + j*q + n at mask[32j + c, 32 + OFFS[r]//4 + n]."""
    cc = np.arange(C).reshape(1, C)
    m = np.zeros((128, SHARD // NGRP), dtype=np.float32)
    for r, (off, sz) in enumerate(zip(OFFS, CHUNKS)):
        q = sz // NGRP
        moff = off // NGRP
        for j in range(NGRP):
            lab = labels_sh[off + j * q : off + (j + 1) * q].astype(np.int64)
            ll = lab.reshape(q, 1)
            opp = (cc < MID) != (ll < MID)
            w = np.where(cc == ll, 0.0, np.where(opp, OPP_W, 1.0))  # [q, C]
            m[32 * j : 32 * j + C, moff : moff + q] = w.T
            m[32 * j + C, moff : moff + q] = -float(C + MID - 1)
    cp = np.concatenate([uzw4, m], axis=1)
    return cp.astype(ml_dtypes.bfloat16)


def _pack_consts(cpack_bf16, wq_fp8, bias4_f32) -> np.ndarray:
    """One [128, CPACK_B+WQ_B+4] uint8 tensor: cpack | wq | bias4."""
    out = np.concatenate(
        [
            np.ascontiguousarray(cpack_bf16).view(np.uint8),
            np.ascontiguousarray(wq_fp8).view(np.uint8),
            np.ascontiguousarray(bias4_f32).view(np.uint8),
        ],
        axis=1,
    )
    return out


def _prepare_reps(reps_sh: np.ndarray) -> np.ndarray:
    """repsq[p, KCH*off + k*sz + m] = fp8(reps_sh[off + m, 128k + p])
    for each chunk (off, sz)."""
    out = np.empty((128, KCH * SHARD), dtype=ml_dtypes.float8_e4m3)
    for off, sz in zip(OFFS, CHUNKS):
        x = reps_sh[off : off + sz].astype(ml_dtypes.float8_e4m3)
        x = x.reshape(sz, KCH, 128)                 # [m, k, p]
        x = np.ascontiguousarray(x.transpose(2, 1, 0))  # [p, k, m]
        out[:, KCH * off : KCH * (off + sz)] = x.reshape(128, KCH * sz)
    return out


def kernel(reps, W, b, labels):
    from concourse.bass_utils import run_bass_kernel_spmd

    reps = np.asarray(reps, dtype=np.float32)
    W = np.asarray(W, dtype=np.float32)
    b = np.asarray(b, dtype=np.float32)
    labels_np = np.asarray(labels)

    if "nc" not in _CACHE:
        _CACHE["nc"] = _build_nc()
    nc = _CACHE["nc"]

    wq, uzw4, bias4 = _prepare_static(W, b)

    in_maps = []
    for core in range(NCORES):
        sh = slice(core * SHARD, (core + 1) * SHARD)
        in_maps.append(
            {
                "repsq": _prepare_reps(reps[sh]),
                "consts": _pack_consts(
                    _prepare_cpack(uzw4, labels_np[sh]), wq, bias4
                ),
            }
        )

    trace = bool(int(os.environ.get("CC_KERNEL_TRACE", "0")))
    res = run_bass_kernel_spmd(
        nc, in_maps, core_ids=list(range(NCORES)), trace=trace
    )
    if trace:
        _CACHE["last_results"] = res

    total = np.float64(0.0)
    for core in range(NCORES):
        total += np.float64(res.results[core]["partials"].sum(dtype=np.float64))
    loss = -(total / B)
    return np.float32(loss)

